# revision 1
# baseline (speedup 1.0000x reference)
"""ConvLSTM (2-layer, HID=64, 64x64, T=16, B=16) Trainium2 Bass kernel, v4.

Sharding: data-parallel over batch B=16 -> 2 per NeuronCore across 8 cores;
weights/biases replicated; the sequential T-loop runs locally per core.

v4 over v3:
- h0 duplicate maintained by GpSimd tensor_copy (idle engine) instead of
  SBUF->SBUF DMA: the strided 128B-row DMA was packet-storming and the
  og-matmuls stalled ~1us per group start waiting for it.
- elementwise restructure: the if-gate block is stored [f; i] (swapped at
  weight-prep time) and c lives at partitions 0:64 with tanh(g) scratch at
  64:128 of the same tile, so f*c and i*tanh(g) fuse into ONE [128]-lane
  DVE mul; c_new = m12[0:64] + m12[64:128]. DVE: 3 ops/group (was 4).
- head conv interleaved into the last timestep's cell1 elementwise.
- 12-matmul warmup burst after the weight DMAs keeps the PE HAM clock
  warm before the first real matmuls (saves the ~25us cold-clock ramp).

Matmul structure (all bf16, PSUM fp32):
- cell0: K=64 h-part (9 shift-offsets) + K=9 im2col x-part; if-gates on
  array rows 0:64 (rhs = h0 in inp1[0:64]), og-gates on rows 64:128
  (rhs = h0 dup at partitions 64:128) -> concurrent row-tile pairs.
- cell1: K=128 (rhs = [h0; h1] = inp1), M=128 if/og, 9 offsets.
"""
import numpy as np
import ml_dtypes
import concourse.tile as tile
from concourse import mybir, bacc
from concourse.bass import _add_dep_helper
from concourse.bass_utils import run_bass_kernel_spmd

F32 = mybir.dt.float32
BF16 = mybir.dt.bfloat16
SIG = mybir.ActivationFunctionType.Sigmoid
TANH = mybir.ActivationFunctionType.Tanh
RELU = mybir.ActivationFunctionType.Relu

N_CORES = 8
B_LOC = 2
H = W = 64
HP = WP = 66
EG_ROWS = 16
CH_ROWS = 8
N_EG = H // EG_ROWS
N_MM = CH_ROWS * W  # 512


def _build(T=16):
    nc = bacc.Bacc("TRN2", target_bir_lowering=False, debug=False, num_devices=N_CORES)

    x9_d = nc.dram_tensor("x9", [T, 9, B_LOC, H, W], BF16, kind="ExternalInput").ap()
    w0h_d = nc.dram_tensor("w0h", [128, 9, 128], BF16, kind="ExternalInput").ap()
    w0x_d = nc.dram_tensor("w0x", [128, 128], BF16, kind="ExternalInput").ap()
    w1_d = nc.dram_tensor("w1t", [128, 2, 9, 128], BF16, kind="ExternalInput").ap()
    b_d = nc.dram_tensor("bt", [128, 4], F32, kind="ExternalInput").ap()
    wh_d = nc.dram_tensor("wht", [128, 1], BF16, kind="ExternalInput").ap()
    bh_d = nc.dram_tensor("bht", [1, 1], F32, kind="ExternalInput").ap()
    y_d = nc.dram_tensor("y", [B_LOC, H * W], F32, kind="ExternalOutput").ap()

    with tile.TileContext(nc) as tc:
        with tc.tile_pool(name="state", bufs=1) as state, \
                tc.tile_pool(name="xq", bufs=2) as xq, \
                tc.tile_pool(name="work", bufs=2) as work, \
                tc.tile_pool(name="psp", bufs=2, space="PSUM") as psp:
            inp1 = state.tile([128, B_LOC, HP, WP], BF16)
            dupB = state.tile([128, B_LOC, HP, WP], BF16)
            c0t = state.tile([128, B_LOC, H, W], F32)
            c1t = state.tile([128, B_LOC, H, W], F32)
            w0h = state.tile([128, 9, 128], BF16)
            w0x = state.tile([128, 128], BF16)
            w1t = state.tile([128, 2, 9, 128], BF16)
            b_sb = state.tile([128, 4], F32)
            whT = state.tile([128, 1], BF16)
            bh_sb = state.tile([1, 1], F32)

            nc.sync.dma_start(out=w1t, in_=w1_d)
            nc.sync.dma_start(out=w0h, in_=w0h_d)
            nc.sync.dma_start(out=w0x, in_=w0x_d)
            nc.sync.dma_start(out=b_sb, in_=b_d)
            nc.sync.dma_start(out=whT, in_=wh_d)
            nc.sync.dma_start(out=bh_sb, in_=bh_d)

            # HAM warmup: keep the PE busy (~6us) so the clock ungates
            # before the first real matmuls; results are never read.
            pw = psp.tile([128, N_MM], F32, tag="pif", name="warm")
            w1f = w1t.rearrange("p a b c -> p (a b c)")
            for i in range(12):
                nc.tensor.matmul(pw, lhsT=w1t[:, 0, 0], rhs=w1f[:, 0:N_MM],
                                 start=(i == 0), stop=(i == 11))

            nc.vector.memset(inp1, 0.0)
            nc.gpsimd.memset(dupB, 0.0)
            nc.vector.memset(c0t[0:64], 0.0)
            nc.vector.memset(c1t[0:64], 0.0)

            h0w = {}
            h1w = {}
            dupw = {}
            mm0_lasts = {}
            mm1_lasts = {}

            def cell0_conv(t, x9t):
                for b in range(B_LOC):
                    for eg in range(N_EG):
                        lasts = mm0_lasts.setdefault((t, b, eg), [])
                        p_if = psp.tile([128, 2, N_MM], F32, tag="pif",
                                        name=f"pif_{t}_0_{b}_{eg}")
                        p_og = psp.tile([128, 2, N_MM], F32, tag="pog",
                                        name=f"pog_{t}_0_{b}_{eg}")
                        for off in range(9):
                            dy, dx = off // 3, off % 3
                            st = off == 0
                            # same-lhsT matmuls back-to-back; LDW for the
                            # other row-group hides under the streaming MM
                            for half in range(2):
                                r0 = eg * EG_ROWS + half * CH_ROWS
                                rif = inp1[0:64, b, r0 + dy:r0 + dy + CH_ROWS, dx:dx + W]
                                mi = nc.tensor.matmul(p_if[:, half], lhsT=w0h[0:64, off],
                                                      rhs=rif, start=st, stop=False)
                                if half == 0 and dy == 0 and (t - 1, b, eg - 1) in h0w:
                                    _add_dep_helper(mi.ins, h0w[(t - 1, b, eg - 1)],
                                                    reason="c0 h0 seam RAW dn")
                                if half == 1 and dy == 2 and (t - 1, b, eg + 1) in h0w:
                                    _add_dep_helper(mi.ins, h0w[(t - 1, b, eg + 1)],
                                                    reason="c0 h0 seam RAW up")
                            for half in range(2):
                                r0 = eg * EG_ROWS + half * CH_ROWS
                                rog = dupB[64:128, b, r0 + dy:r0 + dy + CH_ROWS, dx:dx + W]
                                mo = nc.tensor.matmul(p_og[:, half], lhsT=w0h[64:128, off],
                                                      rhs=rog, start=st, stop=False)
                                if half == 0 and dy == 0 and (t - 1, b, eg - 1) in dupw:
                                    _add_dep_helper(mo.ins, dupw[(t - 1, b, eg - 1)],
                                                    reason="c0 dup seam RAW dn")
                                if half == 1 and dy == 2 and (t - 1, b, eg + 1) in dupw:
                                    _add_dep_helper(mo.ins, dupw[(t - 1, b, eg + 1)],
                                                    reason="c0 dup seam RAW up")
                        for half in range(2):
                            ru = eg * EG_ROWS + half * CH_ROWS
                            mi = nc.tensor.matmul(p_if[:, half], lhsT=w0x[0:9],
                                                  rhs=x9t[0:9, b, ru:ru + CH_ROWS, 0:W],
                                                  start=False, stop=True)
                            lasts.append(mi.ins)
                        for half in range(2):
                            ru = eg * EG_ROWS + half * CH_ROWS
                            mo = nc.tensor.matmul(p_og[:, half], lhsT=w0x[64:73],
                                                  rhs=x9t[64:73, b, ru:ru + CH_ROWS, 0:W],
                                                  start=False, stop=True)
                            lasts.append(mo.ins)
                        yield b, eg, p_if, p_og

            def cell1_conv(t):
                for b in range(B_LOC):
                    for eg in range(N_EG):
                        lasts = mm1_lasts.setdefault((t, b, eg), [])
                        p_if = psp.tile([128, 2, N_MM], F32, tag="pif",
                                        name=f"pif_{t}_1_{b}_{eg}")
                        p_og = psp.tile([128, 2, N_MM], F32, tag="pog",
                                        name=f"pog_{t}_1_{b}_{eg}")
                        for off in range(9):
                            dy, dx = off // 3, off % 3
                            st, sp = off == 0, off == 8
                            for gp, pt in ((0, p_if), (1, p_og)):
                                for half in range(2):
                                    r0 = eg * EG_ROWS + half * CH_ROWS
                                    rhs = inp1[0:128, b, r0 + dy:r0 + dy + CH_ROWS, dx:dx + W]
                                    mm = nc.tensor.matmul(pt[:, half], lhsT=w1t[:, gp, off],
                                                          rhs=rhs, start=st, stop=sp)
                                    if sp:
                                        lasts.append(mm.ins)
                                    if half == 0 and dy == 0 and (t, b, eg - 1) in h0w:
                                        _add_dep_helper(mm.ins, h0w[(t, b, eg - 1)],
                                                        reason="c1 h0 seam RAW dn")
                                    if half == 1 and dy == 2 and (t, b, eg + 1) in h0w:
                                        _add_dep_helper(mm.ins, h0w[(t, b, eg + 1)],
                                                        reason="c1 h0 seam RAW up")
                        yield b, eg, p_if, p_og

            def head_chunk(b, ch, t):
                p_h = psp.tile([1, N_MM], F32, tag="pif", name=f"ph_{b}_{ch}")
                rhs = inp1[64:128, b, 1 + ch * CH_ROWS:1 + (ch + 1) * CH_ROWS, 1:1 + W]
                mh = nc.tensor.matmul(p_h, lhsT=whT[64:128], rhs=rhs,
                                      start=True, stop=True)
                _add_dep_helper(mh.ins, h1w[(t, b, ch // 2)], reason="head RAW")
                h_out = work.tile([1, N_MM], F32, tag="ho", name=f"ho_{b}_{ch}")
                nc.scalar.activation(out=h_out, in_=p_h, func=RELU,
                                     bias=bh_sb[0:1, 0:1])
                nc.sync.dma_start(out=y_d[b:b + 1, ch * N_MM:(ch + 1) * N_MM],
                                  in_=h_out)

            def ew_tail(cell, t, b, eg, o_h, t5, ctf):
                # deferred tail: t5 = tanh(c_new); h = sig(o)*t5
                nc.scalar.activation(out=t5, in_=ctf[0:64], func=TANH)
                rows = slice(1 + eg * EG_ROWS, 1 + (eg + 1) * EG_ROWS)
                if cell == 0:
                    hseg = inp1[0:64, b, rows, 1:1 + W]
                    hw = nc.vector.tensor_mul(hseg, o_h, t5)
                    h0w[(t, b, eg)] = hw.ins
                    for dg in (-1, 1):
                        if (t, b, eg + dg) in mm0_lasts:
                            for mm in mm0_lasts[(t, b, eg + dg)]:
                                _add_dep_helper(hw.ins, mm, reason="h0 seam WAR")
                    # contiguous full-row copy (2112B/partition packets)
                    dd = nc.sync.dma_start(out=dupB[64:128, b, rows, 0:WP],
                                           in_=inp1[0:64, b, rows, 0:WP])
                    dupw[(t, b, eg)] = dd.ins
                    for dg in (-1, 1):
                        if (t, b, eg + dg) in mm0_lasts:
                            for mm in mm0_lasts[(t, b, eg + dg)]:
                                _add_dep_helper(dd.ins, mm, reason="dup seam WAR")
                else:
                    hseg = inp1[64:128, b, rows, 1:1 + W]
                    hw = nc.vector.tensor_mul(hseg, o_h, t5)
                    h1w[(t, b, eg)] = hw.ins
                    for dg in (-1, 1):
                        if (t, b, eg + dg) in mm1_lasts:
                            for mm in mm1_lasts[(t, b, eg + dg)]:
                                _add_dep_helper(hw.ins, mm, reason="h1 seam WAR")
                    if t == T - 1:
                        head_chunk(b, 2 * eg, t)
                        head_chunk(b, 2 * eg + 1, t)

            def elementwise(cell, t, groups):
                # Software-pipelined: each group's tanh(c_new)+h-write is
                # deferred one group, so the ACT stream never blocks on the
                # DVE c-update mid-chain (a stalled t5 would delay the next
                # group's PSUM-bank-freeing reads and stall the PE).
                bcol = 2 * cell
                ct = c0t if cell == 0 else c1t
                pend = None
                for b, eg, p_if, p_og in groups:
                    pif_f = p_if.rearrange("p a b -> p (a b)")
                    pog_f = p_og.rearrange("p a b -> p (a b)")
                    NE = 2 * N_MM
                    egsl = slice(eg * EG_ROWS * W, (eg + 1) * EG_ROWS * W)
                    ctf = ct[:, b].rearrange("p a b -> p (a b)")[:, egsl]
                    # if_h = [sig(f); sig(i)]  (weights pre-swapped on host)
                    if_h = work.tile([128, NE], F32, tag="ifh", name=f"ifh_{t}_{cell}_{b}_{eg}")
                    o_h = work.tile([64, NE], F32, tag="oh", name=f"oh_{t}_{cell}_{b}_{eg}")
                    m1 = work.tile([128, NE], F32, tag="m1", name=f"m1_{t}_{cell}_{b}_{eg}")
                    m2 = work.tile([128, NE], F32, tag="m2", name=f"m2_{t}_{cell}_{b}_{eg}")
                    t5 = work.tile([64, NE], F32, tag="t5", name=f"t5_{t}_{cell}_{b}_{eg}")
                    nc.scalar.activation(out=if_h, in_=pif_f, func=SIG,
                                         bias=b_sb[:, bcol:bcol + 1])
                    # tanh(g) -> scratch at ct[64:128] (base-64, pairs with sig(i))
                    nc.scalar.activation(out=ctf[64:128], in_=pog_f[64:128], func=TANH,
                                         bias=b_sb[64:128, bcol + 1:bcol + 2])
                    nc.scalar.activation(out=o_h, in_=pog_f[0:64], func=SIG,
                                         bias=b_sb[0:64, bcol + 1:bcol + 2])
                    # m1 = f*c (base-0 ins); m2 = i*tanh(g) (base-64 ins)
                    nc.vector.tensor_mul(m1[64:128], if_h[0:64], ctf[0:64])
                    nc.vector.tensor_mul(m2[64:128], if_h[64:128], ctf[64:128])
                    nc.vector.tensor_add(ctf[0:64], m1[64:128], m2[64:128])
                    if pend is not None:
                        ew_tail(cell, t, *pend)
                    pend = (b, eg, o_h, t5, ctf)
                if pend is not None:
                    ew_tail(cell, t, *pend)

            for t in range(T):
                x9t = xq.tile([128, B_LOC, H, W], BF16, tag="x9", name=f"x9_{t}")
                nc.sync.dma_start(out=x9t[0:9], in_=x9_d[t])
                nc.sync.dma_start(out=x9t[64:73], in_=x9_d[t])
                g0 = list(cell0_conv(t, x9t))
                elementwise(0, t, g0)
                g1 = list(cell1_conv(t))
                elementwise(1, t, g1)

    nc.compile()
    return nc


def _prep_inputs(x, w0, b0, w1, b1, wh, bh):
    bf = ml_dtypes.bfloat16
    x = np.asarray(x, np.float32)
    B, T = x.shape[0], x.shape[1]
    bl = B // N_CORES

    xp_all = np.zeros((B, T, HP, WP), np.float32)
    xp_all[:, :, 1:1 + H, 1:1 + W] = x[:, :, 0]
    x9_all = np.empty((B, T, 9, H, W), np.float32)
    for dy in range(3):
        for dx in range(3):
            x9_all[:, :, 3 * dy + dx] = xp_all[:, :, dy:dy + H, dx:dx + W]
    x9_all = x9_all.astype(bf)

    # gate-channel order within the "if" block is swapped to [f; i] so the
    # state update can fuse f*c and i*tanh(g) into one 128-lane mul.
    def swap_if(m_block):
        # m_block [..., 128] over gate channels [i(0:64); f(64:128)]
        return np.concatenate([m_block[..., 64:128], m_block[..., 0:64]], axis=-1)

    w0 = np.asarray(w0, np.float32)  # [256, 65, 3, 3]
    w0h = np.empty((128, 9, 128), np.float32)
    w0x = np.zeros((128, 128), np.float32)
    for dy in range(3):
        for dx in range(3):
            off = 3 * dy + dx
            w0h[0:64, off, :] = swap_if(w0[0:128, 1:65, dy, dx].T)
            w0h[64:128, off, :] = w0[128:256, 1:65, dy, dx].T
            w0x[off, :] = swap_if(w0[0:128, 0, dy, dx])
            w0x[64 + off, :] = w0[128:256, 0, dy, dx]
    w0h = w0h.astype(bf)
    w0x = w0x.astype(bf)

    w1 = np.asarray(w1, np.float32).reshape(2, 128, 128, 3, 3)
    w1t = np.transpose(w1, (2, 0, 3, 4, 1)).reshape(128, 2, 9, 128)
    w1t = w1t.copy()
    w1t[:, 0, :, :] = swap_if(w1t[:, 0, :, :])
    w1t = np.ascontiguousarray(w1t).astype(bf)

    b0 = np.asarray(b0, np.float32)
    b1 = np.asarray(b1, np.float32)
    bif0 = np.concatenate([b0[64:128], b0[0:64]])
    bif1 = np.concatenate([b1[64:128], b1[0:64]])
    bt = np.stack([bif0, b0[128:256], bif1, b1[128:256]], axis=1).astype(np.float32)
    wht = np.zeros((128, 1), np.float32)
    wht[64:128, 0] = np.asarray(wh, np.float32).reshape(64)
    wht = wht.astype(bf)
    bht = np.array([[float(np.asarray(bh).reshape(-1)[0])]], np.float32)

    in_maps = []
    for c in range(N_CORES):
        x9c = np.ascontiguousarray(
            x9_all[c * bl:(c + 1) * bl].transpose(1, 2, 0, 3, 4))
        in_maps.append({"x9": x9c, "w0h": w0h, "w0x": w0x, "w1t": w1t,
                        "bt": bt, "wht": wht, "bht": bht})
    return in_maps


_NC_CACHE = {}


def kernel(x, w0, b0, w1, b1, wh, bh):
    x = np.asarray(x)
    B, T = x.shape[0], x.shape[1]
    if T not in _NC_CACHE:
        _NC_CACHE[T] = _build(T=T)
    nc = _NC_CACHE[T]
    in_maps = _prep_inputs(x, w0, b0, w1, b1, wh, bh)
    res = run_bass_kernel_spmd(nc, in_maps, core_ids=list(range(N_CORES)))
    bl = B // N_CORES
    out = np.zeros((B, 1, H, W), np.float32)
    for c, r in enumerate(res.results):
        out[c * bl:(c + 1) * bl, 0] = r["y"].reshape(bl, H, W)
    return out



# revision 4
# speedup vs baseline: 1.1982x; 1.1982x over previous
"""ConvLSTM (2-layer, HID=64, 64x64, T=16, B=16) Trainium2 Bass kernel, v4.

Sharding: data-parallel over batch B=16 -> 2 per NeuronCore across 8 cores;
weights/biases replicated; the sequential T-loop runs locally per core.

v4 over v3:
- h0 duplicate maintained by GpSimd tensor_copy (idle engine) instead of
  SBUF->SBUF DMA: the strided 128B-row DMA was packet-storming and the
  og-matmuls stalled ~1us per group start waiting for it.
- elementwise restructure: the if-gate block is stored [f; i] (swapped at
  weight-prep time) and c lives at partitions 0:64 with tanh(g) scratch at
  64:128 of the same tile, so f*c and i*tanh(g) fuse into ONE [128]-lane
  DVE mul; c_new = m12[0:64] + m12[64:128]. DVE: 3 ops/group (was 4).
- head conv interleaved into the last timestep's cell1 elementwise.
- 12-matmul warmup burst after the weight DMAs keeps the PE HAM clock
  warm before the first real matmuls (saves the ~25us cold-clock ramp).

Matmul structure (all bf16, PSUM fp32):
- cell0: K=64 h-part (9 shift-offsets) + K=9 im2col x-part; if-gates on
  array rows 0:64 (rhs = h0 in inp1[0:64]), og-gates on rows 64:128
  (rhs = h0 dup at partitions 64:128) -> concurrent row-tile pairs.
- cell1: K=128 (rhs = [h0; h1] = inp1), M=128 if/og, 9 offsets.
"""
import numpy as np
import ml_dtypes
import concourse.tile as tile
from concourse import mybir, bacc
from concourse.bass import _add_dep_helper
from concourse.bass_utils import run_bass_kernel_spmd

F32 = mybir.dt.float32
BF16 = mybir.dt.bfloat16
SIG = mybir.ActivationFunctionType.Sigmoid
TANH = mybir.ActivationFunctionType.Tanh
RELU = mybir.ActivationFunctionType.Relu

N_CORES = 8
B_LOC = 2
H = W = 64
HP = WP = 66
EG_ROWS = 16
CH_ROWS = 8
N_EG = H // EG_ROWS
N_MM = CH_ROWS * W  # 512


def _build(T=16):
    nc = bacc.Bacc("TRN2", target_bir_lowering=False, debug=False, num_devices=N_CORES)

    x9_d = nc.dram_tensor("x9", [T, 9, B_LOC, H, W], BF16, kind="ExternalInput").ap()
    w0h_d = nc.dram_tensor("w0h", [128, 9, 128], BF16, kind="ExternalInput").ap()
    w0x_d = nc.dram_tensor("w0x", [128, 128], BF16, kind="ExternalInput").ap()
    w1_d = nc.dram_tensor("w1t", [128, 2, 9, 128], BF16, kind="ExternalInput").ap()
    b_d = nc.dram_tensor("bt", [128, 4], F32, kind="ExternalInput").ap()
    wh_d = nc.dram_tensor("wht", [128, 1], BF16, kind="ExternalInput").ap()
    bh_d = nc.dram_tensor("bht", [1, 1], F32, kind="ExternalInput").ap()
    y_d = nc.dram_tensor("y", [B_LOC, H * W], F32, kind="ExternalOutput").ap()

    with tile.TileContext(nc) as tc:
        with tc.tile_pool(name="state", bufs=1) as state, \
                tc.tile_pool(name="xq", bufs=2) as xq, \
                tc.tile_pool(name="work", bufs=2) as work, \
                tc.tile_pool(name="psp", bufs=2, space="PSUM") as psp:
            inp1 = state.tile([128, B_LOC, HP, WP], BF16)
            dupB = state.tile([128, B_LOC, HP, WP], BF16)
            c0t = state.tile([128, B_LOC, H, W], F32)
            c1t = state.tile([128, B_LOC, H, W], F32)
            w0h = state.tile([128, 9, 128], BF16)
            w0x = state.tile([128, 128], BF16)
            w1t = state.tile([128, 2, 9, 128], BF16)
            b_sb = state.tile([128, 4], F32)
            whT = state.tile([128, 1], BF16)
            bh_sb = state.tile([1, 1], F32)

            nc.sync.dma_start(out=w1t, in_=w1_d)
            nc.sync.dma_start(out=w0h, in_=w0h_d)
            nc.sync.dma_start(out=w0x, in_=w0x_d)
            nc.sync.dma_start(out=b_sb, in_=b_d)
            nc.sync.dma_start(out=whT, in_=wh_d)
            nc.sync.dma_start(out=bh_sb, in_=bh_d)

            # HAM warmup: keep the PE busy (~6us) so the clock ungates
            # before the first real matmuls; results are never read.
            pw = psp.tile([128, N_MM], F32, tag="pif", name="warm")
            w1f = w1t.rearrange("p a b c -> p (a b c)")
            for i in range(12):
                nc.tensor.matmul(pw, lhsT=w1t[:, 0, 0], rhs=w1f[:, 0:N_MM],
                                 start=(i == 0), stop=(i == 11))

            nc.vector.memset(inp1, 0.0)
            nc.gpsimd.memset(dupB, 0.0)
            nc.vector.memset(c0t[0:64], 0.0)
            nc.vector.memset(c1t[0:64], 0.0)

            h0w = {}
            h1w = {}
            dupw = {}
            mm0_lasts = {}
            mm1_lasts = {}

            def cell0_conv(t, x9t):
                for b in range(B_LOC):
                    for eg in range(N_EG):
                        lasts = mm0_lasts.setdefault((t, b, eg), [])
                        p_if = psp.tile([128, 2, N_MM], F32, tag="pif",
                                        name=f"pif_{t}_0_{b}_{eg}")
                        p_og = psp.tile([128, 2, N_MM], F32, tag="pog",
                                        name=f"pog_{t}_0_{b}_{eg}")
                        for off in range(9):
                            dy, dx = off // 3, off % 3
                            st = off == 0
                            # same-lhsT matmuls back-to-back; LDW for the
                            # other row-group hides under the streaming MM
                            for half in range(2):
                                r0 = eg * EG_ROWS + half * CH_ROWS
                                rif = inp1[0:64, b, r0 + dy:r0 + dy + CH_ROWS, dx:dx + W]
                                mi = nc.tensor.matmul(p_if[:, half], lhsT=w0h[0:64, off],
                                                      rhs=rif, start=st, stop=False)
                                if half == 0 and dy == 0 and (t - 1, b, eg - 1) in h0w:
                                    _add_dep_helper(mi.ins, h0w[(t - 1, b, eg - 1)],
                                                    reason="c0 h0 seam RAW dn")
                                if half == 1 and dy == 2 and (t - 1, b, eg + 1) in h0w:
                                    _add_dep_helper(mi.ins, h0w[(t - 1, b, eg + 1)],
                                                    reason="c0 h0 seam RAW up")
                            for half in range(2):
                                r0 = eg * EG_ROWS + half * CH_ROWS
                                rog = dupB[64:128, b, r0 + dy:r0 + dy + CH_ROWS, dx:dx + W]
                                mo = nc.tensor.matmul(p_og[:, half], lhsT=w0h[64:128, off],
                                                      rhs=rog, start=st, stop=False)
                                if st and half == 0 and (t - 1) in dupw:
                                    _add_dep_helper(mo.ins, dupw[t - 1],
                                                    reason="c0 dup RAW")
                        for half in range(2):
                            ru = eg * EG_ROWS + half * CH_ROWS
                            mi = nc.tensor.matmul(p_if[:, half], lhsT=w0x[0:9],
                                                  rhs=x9t[0:9, b, ru:ru + CH_ROWS, 0:W],
                                                  start=False, stop=True)
                            lasts.append(mi.ins)
                        for half in range(2):
                            ru = eg * EG_ROWS + half * CH_ROWS
                            mo = nc.tensor.matmul(p_og[:, half], lhsT=w0x[64:73],
                                                  rhs=x9t[64:73, b, ru:ru + CH_ROWS, 0:W],
                                                  start=False, stop=True)
                            lasts.append(mo.ins)
                        yield b, eg, p_if, p_og

            def cell1_conv(t):
                for b in range(B_LOC):
                    for eg in range(N_EG):
                        lasts = mm1_lasts.setdefault((t, b, eg), [])
                        p_if = psp.tile([128, 2, N_MM], F32, tag="pif",
                                        name=f"pif_{t}_1_{b}_{eg}")
                        p_og = psp.tile([128, 2, N_MM], F32, tag="pog",
                                        name=f"pog_{t}_1_{b}_{eg}")
                        for off in range(9):
                            dy, dx = off // 3, off % 3
                            st, sp = off == 0, off == 8
                            for gp, pt in ((0, p_if), (1, p_og)):
                                for half in range(2):
                                    r0 = eg * EG_ROWS + half * CH_ROWS
                                    rhs = inp1[0:128, b, r0 + dy:r0 + dy + CH_ROWS, dx:dx + W]
                                    mm = nc.tensor.matmul(pt[:, half], lhsT=w1t[:, gp, off],
                                                          rhs=rhs, start=st, stop=sp)
                                    if sp:
                                        lasts.append(mm.ins)
                                    if half == 0 and dy == 0 and (t, b, eg - 1) in h0w:
                                        _add_dep_helper(mm.ins, h0w[(t, b, eg - 1)],
                                                        reason="c1 h0 seam RAW dn")
                                    if half == 1 and dy == 2 and (t, b, eg + 1) in h0w:
                                        _add_dep_helper(mm.ins, h0w[(t, b, eg + 1)],
                                                        reason="c1 h0 seam RAW up")
                        yield b, eg, p_if, p_og

            def head_chunk(b, ch, t):
                p_h = psp.tile([1, N_MM], F32, tag="pif", name=f"ph_{b}_{ch}")
                rhs = inp1[64:128, b, 1 + ch * CH_ROWS:1 + (ch + 1) * CH_ROWS, 1:1 + W]
                mh = nc.tensor.matmul(p_h, lhsT=whT[64:128], rhs=rhs,
                                      start=True, stop=True)
                _add_dep_helper(mh.ins, h1w[(t, b, ch // 2)], reason="head RAW")
                h_out = work.tile([1, N_MM], F32, tag="ho", name=f"ho_{b}_{ch}")
                nc.scalar.activation(out=h_out, in_=p_h, func=RELU,
                                     bias=bh_sb[0:1, 0:1])
                nc.sync.dma_start(out=y_d[b:b + 1, ch * N_MM:(ch + 1) * N_MM],
                                  in_=h_out)

            def ew_tail(cell, t, b, eg, o_h, t5, ctf):
                # deferred tail: t5 = tanh(c_new); h = sig(o)*t5
                nc.scalar.activation(out=t5, in_=ctf[0:64], func=TANH)
                rows = slice(1 + eg * EG_ROWS, 1 + (eg + 1) * EG_ROWS)
                if cell == 0:
                    hseg = inp1[0:64, b, rows, 1:1 + W]
                    hw = nc.vector.tensor_mul(hseg, o_h, t5)
                    h0w[(t, b, eg)] = hw.ins
                    for dg in (-1, 1):
                        if (t, b, eg + dg) in mm0_lasts:
                            for mm in mm0_lasts[(t, b, eg + dg)]:
                                _add_dep_helper(hw.ins, mm, reason="h0 seam WAR")
                else:
                    hseg = inp1[64:128, b, rows, 1:1 + W]
                    hw = nc.vector.tensor_mul(hseg, o_h, t5)
                    h1w[(t, b, eg)] = hw.ins
                    for dg in (-1, 1):
                        if (t, b, eg + dg) in mm1_lasts:
                            for mm in mm1_lasts[(t, b, eg + dg)]:
                                _add_dep_helper(hw.ins, mm, reason="h1 seam WAR")
                    if t == T - 1:
                        head_chunk(b, 2 * eg, t)
                        head_chunk(b, 2 * eg + 1, t)

            def elementwise(cell, t, groups):
                # Software-pipelined: each group's tanh(c_new)+h-write is
                # deferred one group, so the ACT stream never blocks on the
                # DVE c-update mid-chain (a stalled t5 would delay the next
                # group's PSUM-bank-freeing reads and stall the PE).
                bcol = 2 * cell
                ct = c0t if cell == 0 else c1t
                pend = None
                for b, eg, p_if, p_og in groups:
                    pif_f = p_if.rearrange("p a b -> p (a b)")
                    pog_f = p_og.rearrange("p a b -> p (a b)")
                    NE = 2 * N_MM
                    egsl = slice(eg * EG_ROWS * W, (eg + 1) * EG_ROWS * W)
                    ctf = ct[:, b].rearrange("p a b -> p (a b)")[:, egsl]
                    # if_h = [sig(f); sig(i)]  (weights pre-swapped on host)
                    if_h = work.tile([128, NE], F32, tag="ifh", name=f"ifh_{t}_{cell}_{b}_{eg}")
                    o_h = work.tile([64, NE], F32, tag="oh", name=f"oh_{t}_{cell}_{b}_{eg}")
                    m1 = work.tile([128, NE], F32, tag="m1", name=f"m1_{t}_{cell}_{b}_{eg}")
                    m2 = work.tile([128, NE], F32, tag="m2", name=f"m2_{t}_{cell}_{b}_{eg}")
                    t5 = work.tile([64, NE], F32, tag="t5", name=f"t5_{t}_{cell}_{b}_{eg}")
                    nc.scalar.activation(out=if_h, in_=pif_f, func=SIG,
                                         bias=b_sb[:, bcol:bcol + 1])
                    # tanh(g) -> scratch at ct[64:128] (base-64, pairs with sig(i))
                    nc.scalar.activation(out=ctf[64:128], in_=pog_f[64:128], func=TANH,
                                         bias=b_sb[64:128, bcol + 1:bcol + 2])
                    nc.scalar.activation(out=o_h, in_=pog_f[0:64], func=SIG,
                                         bias=b_sb[0:64, bcol + 1:bcol + 2])
                    # m1 = f*c (base-0 ins); m2 = i*tanh(g) (base-64 ins)
                    nc.vector.tensor_mul(m1[64:128], if_h[0:64], ctf[0:64])
                    nc.vector.tensor_mul(m2[64:128], if_h[64:128], ctf[64:128])
                    nc.vector.tensor_add(ctf[0:64], m1[64:128], m2[64:128])
                    if pend is not None:
                        ew_tail(cell, t, *pend)
                    pend = (b, eg, o_h, t5, ctf)
                if pend is not None:
                    ew_tail(cell, t, *pend)

            for t in range(T):
                x9t = xq.tile([128, B_LOC, H, W], BF16, tag="x9", name=f"x9_{t}")
                nc.sync.dma_start(out=x9t[0:9], in_=x9_d[t])
                nc.sync.dma_start(out=x9t[64:73], in_=x9_d[t])
                g0 = list(cell0_conv(t, x9t))
                elementwise(0, t, g0)
                if t < T - 1:
                    # one bulk h0 dup per step (64 contiguous 17.4KB packets)
                    # instead of 8 per-group strided copies that serialized
                    # the single dynamic DMA queue and landed a timestep late,
                    # stalling every og-group start ~1.7us.
                    dd = nc.scalar.dma_start(out=dupB[64:128], in_=inp1[0:64])
                    dupw[t] = dd.ins
                    for b in range(B_LOC):
                        for eg in range(N_EG):
                            _add_dep_helper(dd.ins, h0w[(t, b, eg)],
                                            reason="dup RAW h0")
                    # WAR: last cell0 MMs of t (PE in-order) read dupB
                    for mm in mm0_lasts[(t, B_LOC - 1, N_EG - 1)]:
                        _add_dep_helper(dd.ins, mm, reason="dup WAR og")
                g1 = list(cell1_conv(t))
                elementwise(1, t, g1)

    nc.compile()
    return nc


def _prep_inputs(x, w0, b0, w1, b1, wh, bh):
    bf = ml_dtypes.bfloat16
    x = np.asarray(x, np.float32)
    B, T = x.shape[0], x.shape[1]
    bl = B // N_CORES

    xp_all = np.zeros((B, T, HP, WP), np.float32)
    xp_all[:, :, 1:1 + H, 1:1 + W] = x[:, :, 0]
    x9_all = np.empty((B, T, 9, H, W), np.float32)
    for dy in range(3):
        for dx in range(3):
            x9_all[:, :, 3 * dy + dx] = xp_all[:, :, dy:dy + H, dx:dx + W]
    x9_all = x9_all.astype(bf)

    # gate-channel order within the "if" block is swapped to [f; i] so the
    # state update can fuse f*c and i*tanh(g) into one 128-lane mul.
    def swap_if(m_block):
        # m_block [..., 128] over gate channels [i(0:64); f(64:128)]
        return np.concatenate([m_block[..., 64:128], m_block[..., 0:64]], axis=-1)

    w0 = np.asarray(w0, np.float32)  # [256, 65, 3, 3]
    w0h = np.empty((128, 9, 128), np.float32)
    w0x = np.zeros((128, 128), np.float32)
    for dy in range(3):
        for dx in range(3):
            off = 3 * dy + dx
            w0h[0:64, off, :] = swap_if(w0[0:128, 1:65, dy, dx].T)
            w0h[64:128, off, :] = w0[128:256, 1:65, dy, dx].T
            w0x[off, :] = swap_if(w0[0:128, 0, dy, dx])
            w0x[64 + off, :] = w0[128:256, 0, dy, dx]
    w0h = w0h.astype(bf)
    w0x = w0x.astype(bf)

    w1 = np.asarray(w1, np.float32).reshape(2, 128, 128, 3, 3)
    w1t = np.transpose(w1, (2, 0, 3, 4, 1)).reshape(128, 2, 9, 128)
    w1t = w1t.copy()
    w1t[:, 0, :, :] = swap_if(w1t[:, 0, :, :])
    w1t = np.ascontiguousarray(w1t).astype(bf)

    b0 = np.asarray(b0, np.float32)
    b1 = np.asarray(b1, np.float32)
    bif0 = np.concatenate([b0[64:128], b0[0:64]])
    bif1 = np.concatenate([b1[64:128], b1[0:64]])
    bt = np.stack([bif0, b0[128:256], bif1, b1[128:256]], axis=1).astype(np.float32)
    wht = np.zeros((128, 1), np.float32)
    wht[64:128, 0] = np.asarray(wh, np.float32).reshape(64)
    wht = wht.astype(bf)
    bht = np.array([[float(np.asarray(bh).reshape(-1)[0])]], np.float32)

    in_maps = []
    for c in range(N_CORES):
        x9c = np.ascontiguousarray(
            x9_all[c * bl:(c + 1) * bl].transpose(1, 2, 0, 3, 4))
        in_maps.append({"x9": x9c, "w0h": w0h, "w0x": w0x, "w1t": w1t,
                        "bt": bt, "wht": wht, "bht": bht})
    return in_maps


_NC_CACHE = {}


def kernel(x, w0, b0, w1, b1, wh, bh):
    x = np.asarray(x)
    B, T = x.shape[0], x.shape[1]
    if T not in _NC_CACHE:
        _NC_CACHE[T] = _build(T=T)
    nc = _NC_CACHE[T]
    in_maps = _prep_inputs(x, w0, b0, w1, b1, wh, bh)
    res = run_bass_kernel_spmd(nc, in_maps, core_ids=list(range(N_CORES)))
    bl = B // N_CORES
    out = np.zeros((B, 1, H, W), np.float32)
    for c, r in enumerate(res.results):
        out[c * bl:(c + 1) * bl, 0] = r["y"].reshape(bl, H, W)
    return out



# revision 7
# speedup vs baseline: 1.1982x; 1.0000x over previous
"""ConvLSTM (2-layer, HID=64, 64x64, T=16, B=16) Trainium2 Bass kernel, v4.

Sharding: data-parallel over batch B=16 -> 2 per NeuronCore across 8 cores;
weights/biases replicated; the sequential T-loop runs locally per core.

v4 over v3:
- h0 duplicate maintained by GpSimd tensor_copy (idle engine) instead of
  SBUF->SBUF DMA: the strided 128B-row DMA was packet-storming and the
  og-matmuls stalled ~1us per group start waiting for it.
- elementwise restructure: the if-gate block is stored [f; i] (swapped at
  weight-prep time) and c lives at partitions 0:64 with tanh(g) scratch at
  64:128 of the same tile, so f*c and i*tanh(g) fuse into ONE [128]-lane
  DVE mul; c_new = m12[0:64] + m12[64:128]. DVE: 3 ops/group (was 4).
- head conv interleaved into the last timestep's cell1 elementwise.
- 12-matmul warmup burst after the weight DMAs keeps the PE HAM clock
  warm before the first real matmuls (saves the ~25us cold-clock ramp).

Matmul structure (all bf16, PSUM fp32):
- cell0: K=64 h-part (9 shift-offsets) + K=9 im2col x-part; if-gates on
  array rows 0:64 (rhs = h0 in inp1[0:64]), og-gates on rows 64:128
  (rhs = h0 dup at partitions 64:128) -> concurrent row-tile pairs.
- cell1: K=128 (rhs = [h0; h1] = inp1), M=128 if/og, 9 offsets.
"""
import numpy as np
import ml_dtypes
import concourse.tile as tile
from concourse import mybir, bacc
from concourse.bass import _add_dep_helper
from concourse.bass_utils import run_bass_kernel_spmd

F32 = mybir.dt.float32
BF16 = mybir.dt.bfloat16
SIG = mybir.ActivationFunctionType.Sigmoid
TANH = mybir.ActivationFunctionType.Tanh
RELU = mybir.ActivationFunctionType.Relu

N_CORES = 8
B_LOC = 2
H = W = 64
HP = WP = 66
EG_ROWS = 16
CH_ROWS = 8
N_EG = H // EG_ROWS
N_MM = CH_ROWS * W  # 512


def _build(T=16):
    nc = bacc.Bacc("TRN2", target_bir_lowering=False, debug=False, num_devices=N_CORES)

    x9_d = nc.dram_tensor("x9", [T, 9, B_LOC, H, W], BF16, kind="ExternalInput").ap()
    w0h_d = nc.dram_tensor("w0h", [128, 9, 128], BF16, kind="ExternalInput").ap()
    w0x_d = nc.dram_tensor("w0x", [128, 128], BF16, kind="ExternalInput").ap()
    w1_d = nc.dram_tensor("w1t", [128, 2, 9, 128], BF16, kind="ExternalInput").ap()
    b_d = nc.dram_tensor("bt", [128, 4], F32, kind="ExternalInput").ap()
    wh_d = nc.dram_tensor("wht", [128, 1], BF16, kind="ExternalInput").ap()
    bh_d = nc.dram_tensor("bht", [1, 1], F32, kind="ExternalInput").ap()
    y_d = nc.dram_tensor("y", [B_LOC, H * W], F32, kind="ExternalOutput").ap()

    with tile.TileContext(nc) as tc:
        with tc.tile_pool(name="state", bufs=1) as state, \
                tc.tile_pool(name="xq", bufs=2) as xq, \
                tc.tile_pool(name="work", bufs=2) as work, \
                tc.tile_pool(name="late", bufs=4) as late, \
                tc.tile_pool(name="psp", bufs=2, space="PSUM") as psp:
            inp1 = state.tile([128, B_LOC, HP, WP], BF16)
            dupB = state.tile([128, B_LOC, HP, WP], BF16)
            c0t = state.tile([128, B_LOC, H, W], F32)
            c1t = state.tile([128, B_LOC, H, W], F32)
            w0h = state.tile([128, 9, 128], BF16)
            w0x = state.tile([128, 128], BF16)
            w1t = state.tile([128, 2, 9, 128], BF16)
            b_sb = state.tile([128, 4], F32)
            whT = state.tile([128, 1], BF16)
            bh_sb = state.tile([1, 1], F32)

            nc.sync.dma_start(out=w1t, in_=w1_d)
            nc.sync.dma_start(out=w0h, in_=w0h_d)
            nc.sync.dma_start(out=w0x, in_=w0x_d)
            nc.sync.dma_start(out=b_sb, in_=b_d)
            nc.sync.dma_start(out=whT, in_=wh_d)
            nc.sync.dma_start(out=bh_sb, in_=bh_d)

            # HAM warmup: keep the PE busy (~6us) so the clock ungates
            # before the first real matmuls; results are never read.
            pw = psp.tile([128, N_MM], F32, tag="pif", name="warm")
            w1f = w1t.rearrange("p a b c -> p (a b c)")
            for i in range(12):
                nc.tensor.matmul(pw, lhsT=w1t[:, 0, 0], rhs=w1f[:, 0:N_MM],
                                 start=(i == 0), stop=(i == 11))

            nc.vector.memset(inp1, 0.0)
            nc.gpsimd.memset(dupB, 0.0)
            nc.vector.memset(c0t[0:64], 0.0)
            nc.vector.memset(c1t[0:64], 0.0)

            h0w = {}
            h1w = {}
            dupw = {}
            mm0_lasts = {}
            mm1_lasts = {}

            def cell0_conv(t, x9t):
                for b in range(B_LOC):
                    for eg in range(N_EG):
                        lasts = mm0_lasts.setdefault((t, b, eg), [])
                        p_if = psp.tile([128, 2, N_MM], F32, tag="pif",
                                        name=f"pif_{t}_0_{b}_{eg}")
                        p_og = psp.tile([128, 2, N_MM], F32, tag="pog",
                                        name=f"pog_{t}_0_{b}_{eg}")
                        for off in range(9):
                            dy, dx = off // 3, off % 3
                            st = off == 0
                            # same-lhsT matmuls back-to-back; LDW for the
                            # other row-group hides under the streaming MM
                            for half in range(2):
                                r0 = eg * EG_ROWS + half * CH_ROWS
                                rif = inp1[0:64, b, r0 + dy:r0 + dy + CH_ROWS, dx:dx + W]
                                mi = nc.tensor.matmul(p_if[:, half], lhsT=w0h[0:64, off],
                                                      rhs=rif, start=st, stop=False)
                                if half == 0 and dy == 0 and (t - 1, b, eg - 1) in h0w:
                                    _add_dep_helper(mi.ins, h0w[(t - 1, b, eg - 1)],
                                                    reason="c0 h0 seam RAW dn")
                                if half == 1 and dy == 2 and (t - 1, b, eg + 1) in h0w:
                                    _add_dep_helper(mi.ins, h0w[(t - 1, b, eg + 1)],
                                                    reason="c0 h0 seam RAW up")
                            for half in range(2):
                                r0 = eg * EG_ROWS + half * CH_ROWS
                                rog = dupB[64:128, b, r0 + dy:r0 + dy + CH_ROWS, dx:dx + W]
                                mo = nc.tensor.matmul(p_og[:, half], lhsT=w0h[64:128, off],
                                                      rhs=rog, start=st, stop=False)
                                if st and half == 0 and (t - 1) in dupw:
                                    _add_dep_helper(mo.ins, dupw[t - 1],
                                                    reason="c0 dup RAW")
                        for half in range(2):
                            ru = eg * EG_ROWS + half * CH_ROWS
                            mi = nc.tensor.matmul(p_if[:, half], lhsT=w0x[0:9],
                                                  rhs=x9t[0:9, b, ru:ru + CH_ROWS, 0:W],
                                                  start=False, stop=True)
                            lasts.append(mi.ins)
                        for half in range(2):
                            ru = eg * EG_ROWS + half * CH_ROWS
                            mo = nc.tensor.matmul(p_og[:, half], lhsT=w0x[64:73],
                                                  rhs=x9t[64:73, b, ru:ru + CH_ROWS, 0:W],
                                                  start=False, stop=True)
                            lasts.append(mo.ins)
                        yield b, eg, p_if, p_og

            def cell1_conv(t):
                for b in range(B_LOC):
                    for eg in range(N_EG):
                        lasts = mm1_lasts.setdefault((t, b, eg), [])
                        p_if = psp.tile([128, 2, N_MM], F32, tag="pif",
                                        name=f"pif_{t}_1_{b}_{eg}")
                        p_og = psp.tile([128, 2, N_MM], F32, tag="pog",
                                        name=f"pog_{t}_1_{b}_{eg}")
                        for off in range(9):
                            dy, dx = off // 3, off % 3
                            st, sp = off == 0, off == 8
                            for gp, pt in ((0, p_if), (1, p_og)):
                                for half in range(2):
                                    r0 = eg * EG_ROWS + half * CH_ROWS
                                    rhs = inp1[0:128, b, r0 + dy:r0 + dy + CH_ROWS, dx:dx + W]
                                    mm = nc.tensor.matmul(pt[:, half], lhsT=w1t[:, gp, off],
                                                          rhs=rhs, start=st, stop=sp)
                                    if sp:
                                        lasts.append(mm.ins)
                                    if half == 0 and dy == 0 and (t, b, eg - 1) in h0w:
                                        _add_dep_helper(mm.ins, h0w[(t, b, eg - 1)],
                                                        reason="c1 h0 seam RAW dn")
                                    if half == 1 and dy == 2 and (t, b, eg + 1) in h0w:
                                        _add_dep_helper(mm.ins, h0w[(t, b, eg + 1)],
                                                        reason="c1 h0 seam RAW up")
                        yield b, eg, p_if, p_og

            def head_chunk(b, ch, t):
                p_h = psp.tile([1, N_MM], F32, tag="pif", name=f"ph_{b}_{ch}")
                rhs = inp1[64:128, b, 1 + ch * CH_ROWS:1 + (ch + 1) * CH_ROWS, 1:1 + W]
                mh = nc.tensor.matmul(p_h, lhsT=whT[64:128], rhs=rhs,
                                      start=True, stop=True)
                _add_dep_helper(mh.ins, h1w[(t, b, ch // 2)], reason="head RAW")
                h_out = work.tile([1, N_MM], F32, tag="ho", name=f"ho_{b}_{ch}")
                nc.scalar.activation(out=h_out, in_=p_h, func=RELU,
                                     bias=bh_sb[0:1, 0:1])
                nc.sync.dma_start(out=y_d[b:b + 1, ch * N_MM:(ch + 1) * N_MM],
                                  in_=h_out)

            def ew_tail(cell, t, b, eg, o_h, ctf):
                # deferred tail: t5 = tanh(c_new); h = sig(o)*t5
                t5 = work.tile([64, 2 * N_MM], F32, tag="t5",
                               name=f"t5_{t}_{cell}_{b}_{eg}")
                nc.scalar.activation(out=t5, in_=ctf[0:64], func=TANH)
                rows = slice(1 + eg * EG_ROWS, 1 + (eg + 1) * EG_ROWS)
                if cell == 0:
                    hseg = inp1[0:64, b, rows, 1:1 + W]
                    hw = nc.vector.tensor_mul(hseg, o_h, t5)
                    h0w[(t, b, eg)] = hw.ins
                    for dg in (-1, 1):
                        if (t, b, eg + dg) in mm0_lasts:
                            for mm in mm0_lasts[(t, b, eg + dg)]:
                                _add_dep_helper(hw.ins, mm, reason="h0 seam WAR")
                else:
                    hseg = inp1[64:128, b, rows, 1:1 + W]
                    hw = nc.vector.tensor_mul(hseg, o_h, t5)
                    h1w[(t, b, eg)] = hw.ins
                    for dg in (-1, 1):
                        if (t, b, eg + dg) in mm1_lasts:
                            for mm in mm1_lasts[(t, b, eg + dg)]:
                                _add_dep_helper(hw.ins, mm, reason="h1 seam WAR")
                    if t == T - 1:
                        head_chunk(b, 2 * eg, t)
                        head_chunk(b, 2 * eg + 1, t)

            def elementwise(cell, t, groups):
                # Software-pipelined: each group's tanh(c_new)+h-write tail is
                # deferred DEFER groups. The tail's t5 ACT op waits on the DVE
                # c-update chain; with a shallow deferral the in-order ACT
                # queue blocks on it, delaying the next groups' PSUM-freeing
                # trio and stalling the PE at og-group starts (~1.4us/group
                # measured at depth 1). Depth 3 keeps the trio pacing the PE
                # while tails ride 3 groups behind, never blocking.
                DEFER = 3
                bcol = 2 * cell
                ct = c0t if cell == 0 else c1t
                pend = []
                for b, eg, p_if, p_og in groups:
                    pif_f = p_if.rearrange("p a b -> p (a b)")
                    pog_f = p_og.rearrange("p a b -> p (a b)")
                    NE = 2 * N_MM
                    egsl = slice(eg * EG_ROWS * W, (eg + 1) * EG_ROWS * W)
                    ctf = ct[:, b].rearrange("p a b -> p (a b)")[:, egsl]
                    # if_h = [sig(f); sig(i)]  (weights pre-swapped on host)
                    if_h = work.tile([128, NE], F32, tag="ifh", name=f"ifh_{t}_{cell}_{b}_{eg}")
                    o_h = late.tile([64, NE], F32, tag="oh", name=f"oh_{t}_{cell}_{b}_{eg}")
                    m1 = work.tile([128, NE], F32, tag="m1", name=f"m1_{t}_{cell}_{b}_{eg}")
                    m2 = work.tile([128, NE], F32, tag="m2", name=f"m2_{t}_{cell}_{b}_{eg}")
                    nc.scalar.activation(out=if_h, in_=pif_f, func=SIG,
                                         bias=b_sb[:, bcol:bcol + 1])
                    # tanh(g) -> scratch at ct[64:128] (base-64, pairs with sig(i))
                    nc.scalar.activation(out=ctf[64:128], in_=pog_f[64:128], func=TANH,
                                         bias=b_sb[64:128, bcol + 1:bcol + 2])
                    nc.scalar.activation(out=o_h, in_=pog_f[0:64], func=SIG,
                                         bias=b_sb[0:64, bcol + 1:bcol + 2])
                    # m1 = f*c (base-0 ins); m2 = i*tanh(g) (base-64 ins)
                    nc.vector.tensor_mul(m1[64:128], if_h[0:64], ctf[0:64])
                    nc.vector.tensor_mul(m2[64:128], if_h[64:128], ctf[64:128])
                    nc.vector.tensor_add(ctf[0:64], m1[64:128], m2[64:128])
                    pend.append((b, eg, o_h, ctf))
                    if len(pend) > DEFER:
                        ew_tail(cell, t, *pend.pop(0))
                for p in pend:
                    ew_tail(cell, t, *p)

            for t in range(T):
                x9t = xq.tile([128, B_LOC, H, W], BF16, tag="x9", name=f"x9_{t}")
                nc.sync.dma_start(out=x9t[0:9], in_=x9_d[t])
                nc.sync.dma_start(out=x9t[64:73], in_=x9_d[t])
                g0 = list(cell0_conv(t, x9t))
                elementwise(0, t, g0)
                if t < T - 1:
                    # one bulk h0 dup per step (64 contiguous 17.4KB packets)
                    # instead of 8 per-group strided copies that serialized
                    # the single dynamic DMA queue and landed a timestep late,
                    # stalling every og-group start ~1.7us.
                    dd = nc.scalar.dma_start(out=dupB[64:128], in_=inp1[0:64])
                    dupw[t] = dd.ins
                    for b in range(B_LOC):
                        for eg in range(N_EG):
                            _add_dep_helper(dd.ins, h0w[(t, b, eg)],
                                            reason="dup RAW h0")
                    # WAR: last cell0 MMs of t (PE in-order) read dupB
                    for mm in mm0_lasts[(t, B_LOC - 1, N_EG - 1)]:
                        _add_dep_helper(dd.ins, mm, reason="dup WAR og")
                g1 = list(cell1_conv(t))
                elementwise(1, t, g1)

    nc.compile()
    return nc


def _prep_inputs(x, w0, b0, w1, b1, wh, bh):
    bf = ml_dtypes.bfloat16
    x = np.asarray(x, np.float32)
    B, T = x.shape[0], x.shape[1]
    bl = B // N_CORES

    xp_all = np.zeros((B, T, HP, WP), np.float32)
    xp_all[:, :, 1:1 + H, 1:1 + W] = x[:, :, 0]
    x9_all = np.empty((B, T, 9, H, W), np.float32)
    for dy in range(3):
        for dx in range(3):
            x9_all[:, :, 3 * dy + dx] = xp_all[:, :, dy:dy + H, dx:dx + W]
    x9_all = x9_all.astype(bf)

    # gate-channel order within the "if" block is swapped to [f; i] so the
    # state update can fuse f*c and i*tanh(g) into one 128-lane mul.
    def swap_if(m_block):
        # m_block [..., 128] over gate channels [i(0:64); f(64:128)]
        return np.concatenate([m_block[..., 64:128], m_block[..., 0:64]], axis=-1)

    w0 = np.asarray(w0, np.float32)  # [256, 65, 3, 3]
    w0h = np.empty((128, 9, 128), np.float32)
    w0x = np.zeros((128, 128), np.float32)
    for dy in range(3):
        for dx in range(3):
            off = 3 * dy + dx
            w0h[0:64, off, :] = swap_if(w0[0:128, 1:65, dy, dx].T)
            w0h[64:128, off, :] = w0[128:256, 1:65, dy, dx].T
            w0x[off, :] = swap_if(w0[0:128, 0, dy, dx])
            w0x[64 + off, :] = w0[128:256, 0, dy, dx]
    w0h = w0h.astype(bf)
    w0x = w0x.astype(bf)

    w1 = np.asarray(w1, np.float32).reshape(2, 128, 128, 3, 3)
    w1t = np.transpose(w1, (2, 0, 3, 4, 1)).reshape(128, 2, 9, 128)
    w1t = w1t.copy()
    w1t[:, 0, :, :] = swap_if(w1t[:, 0, :, :])
    w1t = np.ascontiguousarray(w1t).astype(bf)

    b0 = np.asarray(b0, np.float32)
    b1 = np.asarray(b1, np.float32)
    bif0 = np.concatenate([b0[64:128], b0[0:64]])
    bif1 = np.concatenate([b1[64:128], b1[0:64]])
    bt = np.stack([bif0, b0[128:256], bif1, b1[128:256]], axis=1).astype(np.float32)
    wht = np.zeros((128, 1), np.float32)
    wht[64:128, 0] = np.asarray(wh, np.float32).reshape(64)
    wht = wht.astype(bf)
    bht = np.array([[float(np.asarray(bh).reshape(-1)[0])]], np.float32)

    in_maps = []
    for c in range(N_CORES):
        x9c = np.ascontiguousarray(
            x9_all[c * bl:(c + 1) * bl].transpose(1, 2, 0, 3, 4))
        in_maps.append({"x9": x9c, "w0h": w0h, "w0x": w0x, "w1t": w1t,
                        "bt": bt, "wht": wht, "bht": bht})
    return in_maps


_NC_CACHE = {}


def kernel(x, w0, b0, w1, b1, wh, bh):
    x = np.asarray(x)
    B, T = x.shape[0], x.shape[1]
    if T not in _NC_CACHE:
        _NC_CACHE[T] = _build(T=T)
    nc = _NC_CACHE[T]
    in_maps = _prep_inputs(x, w0, b0, w1, b1, wh, bh)
    res = run_bass_kernel_spmd(nc, in_maps, core_ids=list(range(N_CORES)))
    bl = B // N_CORES
    out = np.zeros((B, 1, H, W), np.float32)
    for c, r in enumerate(res.results):
        out[c * bl:(c + 1) * bl, 0] = r["y"].reshape(bl, H, W)
    return out

